# revision 5
# baseline (speedup 1.0000x reference)
"""Trainium2 Bass kernel: DiT block with cross-attention (nn_DiTBlock_CrossAttn).

Sharding: pure data-parallel over batch. B=8 -> 8 NeuronCores, no collectives.

Design (v6):
- Token-half software pipeline: every stage after qkv (self-attn, so, ln2, cq,
  cross-attn, co, ln3, w1, w2) is token-separable, so the two 512-token halves
  are emitted interleaved. PE-dense stages (projections, FFN) of half A fill
  the ACT-bound exp windows of half B and vice versa.
- All attention-side projections (qkv/so/cq/ck/cv/co) and the adaLN matmul run
  in fp8 e4m3 with DoubleRow perf mode (weights host-scaled x32, dequant
  folded into eviction scale/bias).
- q/k/cq/v and the exp'd attention weights are stored fp8 (numerically free);
  ex tiles pack key-tile PAIRS [P,2,1024] so PV also contracts 256 keys per
  fp8 DoubleRow matmul. One [P,1024] logits PSUM tile holds BOTH heads of a
  pair for one key tile -> a single [P,1024] exp on ACT.
- FFN stays bf16: fp8 for w1 or w2 (emulated) exceeds the 2e-2 gate.
- ada computed in 3 groups of 2 blocks; LN1 modulate starts after group 0.
- Softmax normalize: reciprocal (DVE) -> partition_broadcast (Pool) -> multiply,
  no DRAM roundtrip.
- Elementwise work placed by engine-occupancy: modulates on Pool/ACT/DVE per
  phase, evictions on DVE or ACT depending on which is idle in that window.
"""
import contextlib

import numpy as np
import ml_dtypes

import concourse.bass as bass
import concourse.tile as tile
import concourse.mybir as mybir
from concourse import bacc
from concourse.bass_utils import run_bass_kernel_spmd
from concourse.masks import make_identity

P = 128
N = 1024            # tokens
NH = 512            # tokens per pipeline half
D = 1024            # hidden
KD = D // P         # 8 feature chunks
NT = N // P         # 8 token tiles
H = 16              # heads
HD = 64             # head dim
S = 256             # context tokens
ST = S // P         # 2
CD = 512            # context dim
CKD = CD // P       # 4
MLP = 4096
MT = MLP // P       # 32
EPS = 1e-6
ASCALE = 0.125      # 1/sqrt(HD)
NCORES = 8
WS = 32.0           # fp8 weight pre-scale (host side)

F32 = mybir.dt.float32
BF16 = mybir.dt.bfloat16
F8 = mybir.dt.float8e4
AF = mybir.ActivationFunctionType
OP = mybir.AluOpType
DR = mybir.MatmulPerfMode.DoubleRow

HALF = [slice(0, NH), slice(NH, N)]


def _wcols(w):
    """[din, dout] dram AP -> [p, ko, dout] (feature-chunked lhsT view)."""
    return w.rearrange("(ko p) f -> p ko f", p=P)


def build_nc(taps=(), upto='full'):
    nc = bacc.Bacc("TRN2", target_bir_lowering=False, debug=False)

    d = {}
    d['xt'] = nc.dram_tensor("xt", [D, N], BF16, kind="ExternalInput").ap()
    d['ctx8'] = nc.dram_tensor("ctx8", [CD, S], F8, kind="ExternalInput").ap()
    d['cT'] = nc.dram_tensor("cT", [P, KD], F32, kind="ExternalInput").ap()
    for nm, sh in [("w_qkv8", [D, 3 * D]), ("w_so8", [D, D]), ("w_cq8", [D, D]),
                   ("w_ck8", [CD, D]), ("w_cv8", [CD, D]), ("w_co8", [D, D])]:
        d[nm] = nc.dram_tensor(nm, sh, F8, kind="ExternalInput").ap()
    for nm, sh in [("w1", [D, MLP]), ("w2", [MLP, D])]:
        d[nm] = nc.dram_tensor(nm, sh, BF16, kind="ExternalInput").ap()
    d['w_ada'] = nc.dram_tensor("w_ada", [D, 6 * D], F8, kind="ExternalInput").ap()
    for nm, w in [("bqkT", 16), ("bsoT", KD), ("bcqT", KD), ("bckT", KD),
                  ("bcoT", KD), ("b1T", MT), ("b2T", KD), ("badaT", 48)]:
        d[nm] = nc.dram_tensor(nm, [P, w], F32, kind="ExternalInput").ap()
    d['bv32'] = nc.dram_tensor("bv32", [D], BF16, kind="ExternalInput").ap()
    d['bcv32'] = nc.dram_tensor("bcv32", [D], BF16, kind="ExternalInput").ap()
    out = nc.dram_tensor("out_x", [D, N], F32, kind="ExternalOutput").ap()

    tap_shapes = {
        "ada": ([P, 48], F32), "h1": ([P, KD, N], F8),
        "q": ([P, KD, N], F8), "k": ([P, KD, N], F8),
        "v65": ([P, NT, H, 65], F8), "saO": ([P, KD, N], F8),
        "x2": ([P, KD, N], BF16), "h2": ([P, KD, N], F8),
        "cq": ([P, KD, N], F8), "ck": ([P, KD, S], BF16),
        "cv65": ([P, ST, H, 65], F8), "caO": ([P, KD, N], F8),
        "x3": ([P, KD, N], BF16), "h3": ([P, KD, N], BF16),
        "xT": ([P, KD, N], BF16), "g": ([P, MT, N], BF16),
    }
    tap_aps = {nm: nc.dram_tensor(f"dbg_{nm}", *tap_shapes[nm], kind="ExternalOutput").ap()
               for nm in taps}

    with tile.TileContext(nc) as tc:
        _emit(nc, tc, d, out, tap_aps, upto)
    nc.compile()
    return nc


def _emit(nc, tc, d, out, tap_aps={}, upto='full'):
    def tap(nm, t):
        if nm in tap_aps:
            nc.sync.dma_start(tap_aps[nm], t[:])

    iws = 1.0 / WS

    _ctr = [0]

    def nm(base):
        _ctr[0] += 1
        return f"{base}_{_ctr[0]}"

    gl = contextlib.ExitStack()
    with gl:
        const = gl.enter_context(tc.tile_pool(name="const", bufs=1))
        probe_p = gl.enter_context(tc.tile_pool(name="probe_p", bufs=1))

        def stage_out():
            pr = probe_p.tile([P, 512], F32, tag="probe")
            nc.vector.memset(pr[:], 1.0)
            nc.sync.dma_start(out[0:P, 0:512], pr[:])
        resid = gl.enter_context(tc.tile_pool(name="resid", bufs=2))
        actbf = gl.enter_context(tc.tile_pool(name="actbf", bufs=1))
        actf8 = gl.enter_context(tc.tile_pool(name="actf8", bufs=3))
        act8 = gl.enter_context(tc.tile_pool(name="act8", bufs=3))

        # ---------- constants ----------
        ident = const.tile([P, P], F32, tag="ident")
        make_identity(nc, ident)
        onesD_mat = const.tile([P, P], BF16, tag="onesD_mat")
        nc.vector.memset(onesD_mat[:], 1.0 / D)
        eps_t = const.tile([P, 1], F32, tag="eps")
        nc.vector.memset(eps_t[:], EPS)

        # silu(c) first (tiny DMA; unblocks the ada matmuls)
        ctile = const.tile([P, KD], F32, tag="ctile")
        nc.sync.dma_start(ctile[:], d['cT'])
        # [P, KD, 16]: 16-wide last dim keeps the Ko stride 16B-aligned for
        # DoubleRow's stationary-AP constraint; only column 0 is used.
        silu_cT = const.tile([P, KD, 16], F8, tag="silu_cT")
        nc.scalar.activation(silu_cT[:, :, 0], ctile[:], AF.Silu)

        # x (feature-major, bf16) -- per-chunk DMAs so LN1 stats can start
        # early; issued before the bias tiles so they don't sit behind ten
        # small transfers in the queue
        xT = resid.tile([P, KD, N], BF16, tag="resid", name="xT")
        xt_cols = d['xt'].rearrange("(ko p) n -> p ko n", p=P)
        for k in range(KD):
            nc.sync.dma_start(xT[:, k, :], xt_cols[:, k, :])
        # pre-transposed biases straight from DRAM
        bt = {}
        for bn, w in [("bqkT", 16), ("bsoT", KD), ("bcqT", KD), ("bckT", KD),
                      ("bcoT", KD), ("b1T", MT), ("b2T", KD), ("badaT", 48)]:
            bt[bn] = const.tile([P, w], F32, tag=bn, name=bn)
            nc.sync.dma_start(bt[bn][:], d[bn])

        ada = const.tile([P, 48], F32, tag="ada")
        splus = const.tile([P, 24], F32, tag="splus")
        ada4 = ada.rearrange("p (r j) -> p r j", j=4)
        badaT4 = bt['badaT'].rearrange("p (r j) -> p r j", j=4)

        # ---------- LayerNorm helpers (token-sliced) ----------
        def ln_stats(x_in, sq_p, ch_p, ps_ln, ts):
            w = ts.stop - ts.start
            mu_ps = ps_ln.tile([P, w], F32, tag="psw", name=nm("mups"))
            e2_ps = ps_ln.tile([P, w], F32, tag="psw", name=nm("e2ps"))
            for k in range(KD):
                sq = sq_p.tile([P, w], BF16, tag="lnsq", name=nm("lnsq"))
                nc.vector.tensor_mul(sq[:], x_in[:, k, ts], x_in[:, k, ts])
                for sub in range(w // 512):
                    hs = slice(ts.start + sub * 512, ts.start + (sub + 1) * 512)
                    ss = slice(sub * 512, (sub + 1) * 512)
                    nc.tensor.matmul(mu_ps[:, ss], onesD_mat[:], x_in[:, k, hs],
                                     start=(k == 0), stop=(k == KD - 1))
                    nc.tensor.matmul(e2_ps[:, ss], onesD_mat[:], sq[:, ss],
                                     start=(k == 0), stop=(k == KD - 1))
            mu_bf = ch_p.tile([P, w], BF16, tag="lnch2", name=nm("mubf"))
            nc.vector.tensor_copy(mu_bf[:], mu_ps[:])
            rstd = ch_p.tile([P, w], F32, tag="lnch4", name=nm("rstd"))
            nc.vector.tensor_mul(rstd[:], mu_bf[:], mu_bf[:])
            nc.vector.tensor_sub(rstd[:], e2_ps[:], rstd[:])
            nc.scalar.activation(rstd[:], rstd[:], AF.Sqrt, bias=eps_t[:])
            nc.vector.reciprocal(rstd[:], rstd[:])
            rstd_bf = ch_p.tile([P, w], BF16, tag="lnch2", name=nm("rstdbf"))
            nc.vector.tensor_copy(rstd_bf[:], rstd[:])
            return mu_bf, rstd_bf

        def ln_apply(x_in, g, mu_bf, rstd_bf, h_out, tpool, ts, mod_eng="AP"):
            # modulate engine rotates per chunk; pick engines that are idle
            # in the surrounding pipeline window
            w = ts.stop - ts.start
            for k in range(KD):
                t1 = tpool.tile([P, w], BF16, tag="t1", name=nm("t1"))
                nc.vector.tensor_sub(t1[:], x_in[:, k, ts], mu_bf[:])
                nc.vector.tensor_mul(t1[:], t1[:], rstd_bf[:])
                sp = splus[:, g * 8 + k:g * 8 + k + 1]
                sh = ada[:, g * 16 + k:g * 16 + k + 1]
                e = mod_eng[k % len(mod_eng)]
                if e == "P":
                    nc.gpsimd.tensor_scalar(h_out[:, k, ts], t1[:], sp, sh,
                                            OP.mult, OP.add)
                elif e == "V":
                    nc.vector.tensor_scalar(h_out[:, k, ts], t1[:], sp, sh,
                                            OP.mult, OP.add)
                else:
                    nc.scalar.activation(h_out[:, k, ts], t1[:], AF.Identity,
                                         bias=sh, scale=sp)

        # ---------- fp8 DoubleRow projections (token-sliced) ----------
        def proj8(ps_pool, wp, w8cols, kdin, a8, ts, dout, evict,
                  wdt=F8, kstep=2, pmode=DR, blkw=512):
            w = ts.stop - ts.start
            nsub = max(1, w // 512)
            nw = min(512, w)
            kp_n = kdin // kstep
            for blk in range(dout // blkw):
                wb = wp.tile([P, kdin, blkw], wdt, tag="wproj", name=nm("w8"))
                nc.sync.dma_start(wb[:], w8cols[:, :, blk * blkw:(blk + 1) * blkw])
                for t8 in range(blkw // 128):
                    ps = ps_pool.tile([P, w], F32, tag="psw", name=nm("pmm"))
                    for kp in range(kp_n):
                        ks = slice(kstep * kp, kstep * kp + kstep)
                        for sub in range(nsub):
                            hs = slice(ts.start + sub * 512, ts.start + sub * 512 + nw)
                            ss = slice(sub * 512, sub * 512 + nw)
                            nc.tensor.matmul(ps[:, ss],
                                             wb[:, ks, t8 * 128:(t8 + 1) * 128]
                                             if kstep > 1 else
                                             wb[:, ks.start, t8 * 128:(t8 + 1) * 128],
                                             a8[:, ks, hs] if kstep > 1
                                             else a8[:, ks.start, hs],
                                             start=(kp == 0), stop=(kp == kp_n - 1),
                                             perf_mode=pmode)
                    evict(blk * (blkw // 128) + t8, ps)

        def proj_V8(ps_pool, wp, w8cols, kdin, a8, m_tiles, v65t, bias_b):
            kp_n = kdin // 2
            for blk in range(2):
                wb = wp.tile([P, kdin, 512], F8, tag="wproj", name=nm("w8v"))
                nc.sync.dma_start(wb[:], w8cols[:, :, blk * 512:(blk + 1) * 512])
                for i in range(m_tiles):
                    ps = ps_pool.tile([P, 512], F32, tag="psw", name=nm("pv"))
                    for kp in range(kp_n):
                        ks = slice(2 * kp, 2 * kp + 2)
                        nc.tensor.matmul(ps[:], a8[:, ks, i * 128:(i + 1) * 128],
                                         wb[:, ks, :],
                                         start=(kp == 0), stop=(kp == kp_n - 1),
                                         perf_mode=DR)
                    nc.vector.tensor_add(
                        v65t[:, i, blk * 8:(blk + 1) * 8, 0:64],
                        ps.rearrange("p (h e) -> p h e", h=8),
                        bias_b[:, blk * 512:(blk + 1) * 512]
                        .rearrange("p (h e) -> p h e", h=8))
            nc.vector.memset(v65t[:, :, :, 64:65], WS)

        # ---------- attention core (token-sliced half) ----------
        # one [P, 1024] logits PSUM tile = both heads of the pair for one key
        # tile -> single exp; ex tiles pack key-tile pairs [P, 2, 1024] so PV
        # contracts 256 keys per fp8 DoubleRow matmul.
        def attention(q_T, kv_T, v65t, m_tiles, o8, ts, ap, stage_copy):
            expp, arows, rb, ps_lg, ps_pv = ap
            m_pairs = m_tiles // 2
            for hp in range(8):
                pv_t = [ps_pv.tile([65, NH], F32, tag="pv", name=nm("pv"))
                        for _ in range(2)]
                exs = [None] * m_pairs

                def lgexp(kt):
                    if kt % 2 == 0:
                        exs[kt // 2] = expp.tile([P, 2, 2 * NH], F8, tag="ex",
                                                 name=nm("ex"))
                    lg = ps_lg.tile([P, 2 * NH], F32, tag="lg", name=nm("lg"))
                    for idx, off in ((0, 0), (1, 64)):
                        nc.tensor.matmul(lg[:, idx * NH:(idx + 1) * NH],
                                         kv_T[off:off + 64, hp, kt * 128:(kt + 1) * 128],
                                         q_T[off:off + 64, hp, ts],
                                         start=True, stop=True)
                    nc.scalar.activation(exs[kt // 2][:, kt % 2, :], lg[:],
                                         AF.Exp, scale=ASCALE)

                def pvacc(p):
                    for idx in range(2):
                        nc.tensor.matmul(pv_t[idx][:, :],
                                         v65t[:, 2 * p:2 * p + 2, 2 * hp + idx, :],
                                         exs[p][:, :, idx * NH:(idx + 1) * NH],
                                         start=(p == 0), stop=(p == m_pairs - 1),
                                         perf_mode=DR)

                for i in range(m_tiles + 2):
                    if i < m_tiles:
                        lgexp(i)
                    if i >= 2 and i % 2 == 1:
                        pvacc((i - 2) // 2)
                for idx in range(2):
                    pv = pv_t[idx]
                    if stage_copy:
                        pvs = rb.tile([65, NH], F32, tag="pvs", name=nm("pvs"))
                        nc.vector.tensor_copy(pvs[:], pv[:])
                        pv = pvs
                    rec = arows.tile([1, NH], F32, tag="row", name=nm("rec"))
                    nc.vector.reciprocal(rec[:], pv[64:65, :])
                    rbt = rb.tile([64, NH], F32, tag="rbt", name=nm("rbt"))
                    nc.gpsimd.partition_broadcast(rbt[:], rec[:], 64)
                    off = idx * 64
                    nc.vector.tensor_mul(o8[off:off + 64, hp, ts],
                                         pv[0:64, :], rbt[:])

        # ================= phase 0: LN1 stats + ada (3 groups) =================
        ln1 = contextlib.ExitStack()
        lnb1 = ln1.enter_context(tc.tile_pool(name="lnb1", bufs=3))
        lch1 = ln1.enter_context(tc.tile_pool(name="lch1", bufs=4))
        ps_l1 = ln1.enter_context(tc.tile_pool(name="ps_l1", bufs=1, space="PSUM"))
        mu1, rstd1 = ln_stats(xT, lnb1, lch1, ps_l1, slice(0, N))

        ada_es = contextlib.ExitStack()
        with ada_es:
            adap = ada_es.enter_context(tc.tile_pool(name="adap", bufs=2))
            wadap = ada_es.enter_context(tc.tile_pool(name="wadap", bufs=2))
            ps_ada = ada_es.enter_context(tc.tile_pool(name="ps_ada", bufs=2, space="PSUM"))
            ps_tr = ada_es.enter_context(tc.tile_pool(name="ps_tr", bufs=2, space="PSUM"))
            wada_cols = _wcols(d['w_ada'])
            for grp in range(3):
                # per-group [4, 512] row tile keeps base_partition 0 for the
                # PE transpose
                adarow = adap.tile([4, 512], F32, tag="adarow", name=nm("adarow"))
                for bi, blk in enumerate((2 * grp, 2 * grp + 1)):
                    wb = wadap.tile([P, KD, 1024], F8, tag="wada", name=nm("wada"))
                    nc.sync.dma_start(wb[:], wada_cols[:, :, blk * 1024:(blk + 1) * 1024])
                    for tb in range(2):
                        ps = ps_ada.tile([1, 512], F32, tag="psada", name=nm("psada"))
                        for kp in range(KD // 2):
                            ks = slice(2 * kp, 2 * kp + 2)
                            nc.tensor.matmul(ps[:], silu_cT[:, ks, 0:1],
                                             wb[:, ks, tb * 512:(tb + 1) * 512],
                                             start=(kp == 0), stop=(kp == KD // 2 - 1),
                                             perf_mode=DR)
                        r = bi * 2 + tb
                        ast = adap.tile([1, 512], F32, tag="ast", name=nm("ast"))
                        nc.scalar.activation(ast[:], ps[:], AF.Copy, scale=iws)
                        nc.sync.dma_start(adarow[r:r + 1, :], ast[:])
                # rows 4g..4g+3 complete -> transpose this group, finish its
                # ada/splus slice so LN applies for group g can start
                tp = ps_tr.tile([P, 16], F32, tag="ptr", name=nm("ptr"))
                for j in range(4):
                    nc.tensor.transpose(tp[:, j * 4:(j + 1) * 4],
                                        adarow[:, j * 128:(j + 1) * 128],
                                        ident[0:4, 0:4])
                for j in range(4):
                    nc.vector.tensor_copy(ada4[:, 4 * grp:4 * grp + 4, j],
                                          tp[:, j * 4:(j + 1) * 4])
                nc.vector.tensor_add(ada4[:, 4 * grp:4 * grp + 4, :],
                                     ada4[:, 4 * grp:4 * grp + 4, :],
                                     badaT4[:, 4 * grp:4 * grp + 4, :])
                nc.vector.tensor_scalar_add(
                    splus[:, grp * 8:(grp + 1) * 8],
                    ada[:, grp * 16 + 8:grp * 16 + 16], 1.0)
        tap("ada", ada)
        tap("xT", xT)

        # deferred non-critical loads
        ctx8 = const.tile([P, CKD, S], F8, tag="ctx8")
        nc.sync.dma_start(ctx8[:], d['ctx8'].rearrange("(ko p) n -> p ko n", p=P))
        vbias = const.tile([P, D], BF16, tag="vbias")
        nc.sync.dma_start(vbias[:], d['bv32'][None, :].partition_broadcast(P))
        cvbias = const.tile([P, D], BF16, tag="cvbias")
        nc.sync.dma_start(cvbias[:], d['bcv32'][None, :].partition_broadcast(P))

        # ================= LN1 apply -> h1 (fp8, full tokens) =================
        h1 = act8.tile([P, KD, N], F8, tag="a8", name="h1")
        lnt1 = contextlib.ExitStack()
        with lnt1:
            tp1 = lnt1.enter_context(tc.tile_pool(name="lnt", bufs=2))
            ln_apply(xT, 0, mu1, rstd1, h1, tp1, slice(0, N), mod_eng="AP")
        ln1.close()
        tap("h1", h1)
        if upto == 'ada':
            stage_out()
            return

        # fold so-bias into the residual (xT last read by ln_apply above)
        for k in range(KD):
            nc.vector.tensor_scalar_add(xT[:, k, :], xT[:, k, :], bt['bsoT'][:, k:k + 1])

        # ================= qkv (full tokens) =================
        vp = gl.enter_context(tc.tile_pool(name="vp", bufs=1))
        v65 = vp.tile([P, NT, H, 65], F8, tag="v65")

        qkv_ps = contextlib.ExitStack()
        ps_mm = qkv_ps.enter_context(tc.tile_pool(name="ps_mm", bufs=4, space="PSUM"))
        wq_p = qkv_ps.enter_context(tc.tile_pool(name="wq_p", bufs=2))
        wq_cols = _wcols(d['w_qkv8'])
        proj_V8(ps_mm, wq_p, wq_cols[:, :, 2 * D:3 * D], KD, h1, NT, v65, vbias)

        qT = actf8.tile([P, KD, N], F8, tag="af8", name="qT")
        kT = actf8.tile([P, KD, N], F8, tag="af8", name="kT")

        def ev_qk(t, ps):
            dst = qT if t < 8 else kT
            nc.scalar.activation(dst[:, t % 8, :], ps[:], AF.Identity,
                                 bias=bt['bqkT'][:, t:t + 1], scale=iws)
        proj8(ps_mm, wq_p, wq_cols[:, :, 0:2 * D], KD, h1, slice(0, N),
              2 * D, ev_qk, blkw=1024)
        tap("q", qT); tap("k", kT); tap("v65", v65)
        qkv_ps.close()
        if upto == 'qkv':
            stage_out()
            return

        # ================= pipelined half-token stages =================
        pipe = contextlib.ExitStack()
        with pipe:
            # shared PSUM: attention lg 2x[P,1024] (4 banks) + pv 2x[65,512]
            # (2 banks... bufs=4 for two heads x 2 in flight) + work ring
            ps_lg = pipe.enter_context(tc.tile_pool(name="ps_lg", bufs=2, space="PSUM"))
            ps_pv = pipe.enter_context(tc.tile_pool(name="ps_pv", bufs=2, space="PSUM"))
            ps_wk = pipe.enter_context(tc.tile_pool(name="ps_wk", bufs=2, space="PSUM"))
            expp = pipe.enter_context(tc.tile_pool(name="expp", bufs=4))
            arows = pipe.enter_context(tc.tile_pool(name="arows", bufs=2))
            rb = pipe.enter_context(tc.tile_pool(name="rb", bufs=2))
            wp = pipe.enter_context(tc.tile_pool(name="wp", bufs=2))
            lnb = pipe.enter_context(tc.tile_pool(name="lnb", bufs=2))
            lch = pipe.enter_context(tc.tile_pool(name="lch", bufs=3))
            lnt = pipe.enter_context(tc.tile_pool(name="lnt", bufs=2))
            outst = pipe.enter_context(tc.tile_pool(name="outst", bufs=2))
            ap = (expp, arows, rb, ps_lg, ps_pv)

            kp_ = pipe.enter_context(tc.tile_pool(name="kp", bufs=1))
            vp2 = pipe.enter_context(tc.tile_pool(name="vp2", bufs=1))
            ckT = kp_.tile([P, KD, S], BF16, tag="ckT")
            cv65 = vp2.tile([P, ST, H, 65], F8, tag="cv65")

            saO = act8.tile([P, KD, N], F8, tag="a8", name="saO")
            h2 = act8.tile([P, KD, N], F8, tag="a8", name="h2")
            caO = act8.tile([P, KD, N], F8, tag="a8", name="caO")
            cqT = actf8.tile([P, KD, N], F8, tag="af8", name="cqT")
            h3 = actbf.tile([P, KD, N], BF16, tag="abf", name="h3")
            x2T = resid.tile([P, KD, N], BF16, tag="resid", name="x2T")
            x3T = resid.tile([P, KD, N], BF16, tag="resid", name="x3T")
            gp = pipe.enter_context(tc.tile_pool(name="gp", bufs=1))
            gh = [None, None]

            def SA(half):
                attention(qT, kT, v65, NT, saO, HALF[half], ap, stage_copy=True)

            def CKCV():
                def ev_ck(t, ps):
                    nc.vector.tensor_scalar(ckT[:, t, :], ps[:, 0:S],
                                            iws, bt['bckT'][:, t:t + 1],
                                            OP.mult, OP.add)
                # S=256 tokens -> [P,256] psum per t8 (pad ring to 512 slot)
                proj8(ps_wk, wp, _wcols(d['w_ck8']), CKD, ctx8, slice(0, S),
                      D, ev_ck)
                proj_V8(ps_wk, wp, _wcols(d['w_cv8']), CKD, ctx8, ST, cv65, cvbias)

            def SOLN2CQ(half):
                ts = HALF[half]

                def ev_so(t, ps):
                    nc.vector.scalar_tensor_tensor(x2T[:, t, ts], ps[:], iws,
                                                   xT[:, t, ts], OP.mult, OP.add)
                proj8(ps_wk, wp, _wcols(d['w_so8']), KD, saO, ts, D, ev_so)
                mu, rstd = ln_stats(x2T, lnb, lch, ps_wk, ts)
                ln_apply(x2T, 1, mu, rstd, h2, lnt, ts, mod_eng="PV")
                # fold co-bias into x2T (last read by ln_apply above)
                for k in range(KD):
                    nc.vector.tensor_scalar_add(x2T[:, k, ts], x2T[:, k, ts],
                                                bt['bcoT'][:, k:k + 1])

                def ev_cq(t, ps):
                    nc.scalar.activation(cqT[:, t, ts], ps[:], AF.Identity,
                                         bias=bt['bcqT'][:, t:t + 1], scale=iws)
                proj8(ps_wk, wp, _wcols(d['w_cq8']), KD, h2, ts, D, ev_cq)

            def CA(half):
                attention(cqT, ckT, cv65, ST, caO, HALF[half], ap, stage_copy=False)

            def COLN3(half):
                ts = HALF[half]

                def ev_co(t, ps):
                    nc.vector.scalar_tensor_tensor(x3T[:, t, ts], ps[:], iws,
                                                   x2T[:, t, ts], OP.mult, OP.add)
                proj8(ps_wk, wp, _wcols(d['w_co8']), KD, caO, ts, D, ev_co)
                mu, rstd = ln_stats(x3T, lnb, lch, ps_wk, ts)
                ln_apply(x3T, 2, mu, rstd, h3, lnt, ts, mod_eng="PV")
                # fold b2 into the residual before the final eviction
                for k in range(KD):
                    nc.vector.tensor_scalar_add(x3T[:, k, ts], x3T[:, k, ts],
                                                bt['b2T'][:, k:k + 1])

            def W1(half):
                ts = HALF[half]
                gh[half] = gp.tile([P, MT, NH], BF16, tag="g", name=nm("g"))
                g = gh[half]

                def ev_gelu(t, ps):
                    nc.scalar.activation(g[:, t, :], ps[:], AF.Gelu,
                                         bias=bt['b1T'][:, t:t + 1])
                proj8(ps_wk, wp, _wcols(d['w1']), KD, h3, ts, MLP, ev_gelu,
                      wdt=BF16, kstep=1, pmode=None)

            def W2(half):
                ts = HALF[half]
                g = gh[half]
                w2_cols = d['w2'].rearrange("(ko p) f -> p ko f", p=P)
                for t8 in range(8):
                    wb = wp.tile([P, MT, P], BF16, tag="wproj", name=nm("w2b"))
                    nc.sync.dma_start(wb[:], w2_cols[:, :, t8 * 128:(t8 + 1) * 128])
                    pso = ps_wk.tile([P, NH], F32, tag="psw", name=nm("po"))
                    for m in range(MT):
                        nc.tensor.matmul(pso[:], wb[:, m, :], g[:, m, :],
                                         start=(m == 0), stop=(m == MT - 1))
                    ost = outst.tile([P, NH], F32, tag="ost", name=nm("ost"))
                    nc.vector.tensor_add(ost[:], pso[:], x3T[:, t8, ts])
                    nc.sync.dma_start(out[t8 * 128:(t8 + 1) * 128, ts], ost[:])

            SA(0)
            CKCV()
            SOLN2CQ(0)
            SA(1)
            tap("saO", saO)
            CA(0)
            SOLN2CQ(1)
            tap("x2", x2T); tap("h2", h2); tap("cq", cqT)
            tap("ck", ckT); tap("cv65", cv65)
            COLN3(0)
            CA(1)
            tap("caO", caO)
            W1(0)
            COLN3(1)
            tap("x3", x3T); tap("h3", h3)
            W2(0)
            W1(1)
            W2(1)


_NC = None


def _get_nc():
    global _NC
    if _NC is None:
        _NC = build_nc()
    return _NC


def make_in_maps(inputs):
    f8 = ml_dtypes.float8_e4m3
    ws = WS
    bf = ml_dtypes.bfloat16
    f32 = np.float32
    shared = {}
    for src, dst in [("w_qkv", "w_qkv8"), ("w_so", "w_so8"), ("w_cq", "w_cq8"),
                     ("w_ck", "w_ck8"), ("w_cv", "w_cv8"), ("w_co", "w_co8")]:
        shared[dst] = np.ascontiguousarray(
            (np.asarray(inputs[src], f32) * ws).astype(f8))
    for nm in ("w1", "w2"):
        shared[nm] = np.ascontiguousarray(np.asarray(inputs[nm]).astype(bf))
    shared['w_ada'] = np.ascontiguousarray(
        (np.asarray(inputs['w_ada'], f32) * ws).astype(f8))
    bq = np.asarray(inputs['b_qkv'], f32)
    # bcqT/bckT are prescaled by ws (DVE evicts (ps + b*ws) * iws);
    # bqkT is not (ACT evicts identity(ps*iws + b))
    shared['bqkT'] = np.ascontiguousarray(bq[:2 * D].reshape(16, P).T)
    shared['bv32'] = np.ascontiguousarray((ws * bq[2 * D:]).astype(bf))
    shared['bcv32'] = np.ascontiguousarray(
        (ws * np.asarray(inputs['b_cv'], f32)).astype(bf))
    for src, dst, w, sc in [("b_so", "bsoT", KD, 1.0), ("b_cq", "bcqT", KD, 1.0),
                            ("b_ck", "bckT", KD, 1.0), ("b_co", "bcoT", KD, 1.0),
                            ("b1", "b1T", MT, 1.0), ("b2", "b2T", KD, 1.0),
                            ("b_ada", "badaT", 48, 1.0)]:
        shared[dst] = np.ascontiguousarray(
            sc * np.asarray(inputs[src], f32).reshape(w, P).T)
    x = np.asarray(inputs['x'], f32)
    c = np.asarray(inputs['c'], f32)
    ctxt = np.asarray(inputs['context'], f32)
    in_maps = []
    for i in range(NCORES):
        m = dict(shared)
        m['xt'] = np.ascontiguousarray(x[i].T.astype(bf))
        m['ctx8'] = np.ascontiguousarray(ctxt[i].T.astype(f8))
        m['cT'] = np.ascontiguousarray(c[i].reshape(KD, P).T)
        in_maps.append(m)
    return in_maps


def kernel(**inputs):
    nc = _get_nc()
    in_maps = make_in_maps(inputs)
    res = run_bass_kernel_spmd(nc, in_maps, core_ids=list(range(NCORES)))
    return np.stack([res.results[i]["out_x"].T for i in range(NCORES)]).astype(np.float32)


if __name__ == "__main__":
    data = np.load("/root/problem/inputs.npz")
    out = kernel(**{k: data[k] for k in data.files})
    gold = np.load("/root/problem/gold64.npy")
    err = np.abs(out - gold)
    print("max abs err:", err.max(), " rel:", err.max() / np.abs(gold).max())
